# revision 10
# baseline (speedup 1.0000x reference)
"""Causal multi-head self-attention with RoPE on 8 Trainium2 NeuronCores. v7

Sharding: data parallel over batch (2) x tensor parallel over heads (4 groups
of 4 heads).  Core c handles batch b = c // 4, head group hg = c % 4.

Structure (j-major, pair-phased, chunk-0 predrained):
  - Head pairs (even head on PE row-tile 0, odd on row-tile 1) score
    concurrently via tile_position -> 2x score throughput.
  - RoPE cos/sin tables are synthesized ON DEVICE from an 8KB position row:
    ang = pos * invfreq, range-reduced mod 2pi to [-pi, pi), evaluated with
    the ACT Sin LUT.  The tables come out negated (-sin, -cos), which makes
    qrot/krot globally negated -- the sign cancels in Q.K scores.
  - Both pairs' chunk-0 scores, P^T V and the chunk-0 output projection all
    run early (pair 1's chunk-0 exp lives in a small side buffer), so output
    DMA streams from ~25us and the pair boundary has no ACT bubble.
  - Projections run as dt-outer chains (one weight load per two streams).
  - Softmax denominator: ones column in V_aug; reciprocal broadcast by a
    K=1 fp16 ones-matmul + 64-lane DVE reciprocal.
"""

import numpy as np

import concourse.bass as bass
import concourse.mybir as mybir
import concourse.tile as tile
from concourse import bacc
from concourse.bass_utils import run_bass_kernel_spmd

F32 = mybir.dt.float32
F16 = mybir.dt.float16

B, S, D, H, DH = 2, 2048, 1024, 16, 64
ROPE_THETA = 10000.0
NCORE = 8
HPG = 4
P = 128
NKT = S // P     # 16 k-tiles
NQC = S // 512   # 4 query chunks
PI = float(np.pi)

_W = [S - P * j for j in range(NKT)]
_OFF = np.concatenate([[0], np.cumsum(_W)]).astype(int)
EXP_TOT = int(_OFF[-1])  # 17408 fp16 cols -> 34KB/partition per head


def build_program():
    nc = bacc.Bacc(
        "TRN2", target_bir_lowering=False, debug=False, num_devices=NCORE
    )

    xts = nc.dram_tensor("xts", [NQC, P, 8, 512], F16, kind="ExternalInput")
    wq2 = nc.dram_tensor("wq2", [2, P, 8, P], F16, kind="ExternalInput")
    wk2 = nc.dram_tensor("wk2", [2, P, 8, P], F16, kind="ExternalInput")
    wvT = nc.dram_tensor("wvT", [P, 8, 256], F16, kind="ExternalInput")
    woT = nc.dram_tensor("woT", [P, 2, D], F16, kind="ExternalInput")
    posf = nc.dram_tensor("posf", [1, S], F16, kind="ExternalInput")
    invf = nc.dram_tensor("invf", [P, 1], F32, kind="ExternalInput")
    ST = nc.dram_tensor("ST", [P, P], F16, kind="ExternalInput")
    trimask = nc.dram_tensor("trimask", [P, P], F16, kind="ExternalInput")

    outT = nc.dram_tensor("outT", [8, NQC, P, 512], F16, kind="ExternalOutput")

    with tile.TileContext(nc) as tc:
        with (
            tc.tile_pool(name="big", bufs=1) as big,
            tc.tile_pool(name="tmp", bufs=3) as tmp,
            tc.tile_pool(name="psum", bufs=1, space="PSUM") as psum,
            tc.tile_pool(name="outp", bufs=6) as outp,
        ):
            xt_sb = big.tile([P, NQC, 8, 512], F16, tag="xt")
            wq_sb = big.tile([P, 2, 8, P], F16, tag="wq")
            wk_sb = big.tile([P, 2, 8, P], F16, tag="wk")
            wv_sb = big.tile([P, 8, 256], F16, tag="wv")
            wo_sb = big.tile([P, 2, D], F16, tag="wo")
            cos_sb = big.tile([P, S], F16, tag="cos")
            sin_sb = big.tile([P, S], F16, tag="sin")
            pos_sb = big.tile([1, S], F16, tag="pos")
            invf_sb = big.tile([P, 1], F32, tag="invf")
            st_sb = big.tile([P, P], F16, tag="st")
            tri_sb = big.tile([P, P], F16, tag="tri")
            ones_sb = big.tile([P, 512], F16, tag="ones")
            qrot = big.tile([P, 2, S], F16, tag="qrot")
            krot = big.tile([P, 2, S], F16, tag="krot")
            v_sb = big.tile([P, NKT, HPG, DH + 1], F16, tag="v")
            at_sb = big.tile([P, 2, S], F16, tag="at")
            ep_sb = big.tile([P, 2, EXP_TOT], F16, tag="ep")
            ep1c0 = big.tile([P, 2, 4, 512], F16, tag="ep1c0")

            # ---- input DMA: three queues, ordered by first-use time ----
            nc.scalar.dma_start(out=pos_sb[:], in_=posf[:, :])
            nc.scalar.dma_start(out=invf_sb[:], in_=invf[:, :])
            nc.scalar.dma_start(out=wq_sb[:, 0], in_=wq2[0])
            nc.scalar.dma_start(out=st_sb[:], in_=ST[:, :])
            nc.scalar.dma_start(out=tri_sb[:], in_=trimask[:, :])
            nc.scalar.dma_start(out=wq_sb[:, 1], in_=wq2[1])
            nc.scalar.dma_start(out=wk_sb[:, 1], in_=wk2[1])
            nc.scalar.dma_start(out=wv_sb[:], in_=wvT[:])
            nc.sync.dma_start(out=xt_sb[:, 0, 0:4], in_=xts[0, :, 0:4])
            nc.sync.dma_start(out=wk_sb[:, 0], in_=wk2[0])
            nc.sync.dma_start(out=xt_sb[:, 1, 0:4], in_=xts[1, :, 0:4])
            nc.sync.dma_start(out=xt_sb[:, 2], in_=xts[2])
            nc.sync.dma_start(out=xt_sb[:, 3], in_=xts[3])
            nc.gpsimd.dma_start(out=xt_sb[:, 0, 4:8], in_=xts[0, :, 4:8])
            nc.gpsimd.dma_start(out=xt_sb[:, 1, 4:8], in_=xts[1, :, 4:8])
            nc.gpsimd.dma_start(out=wo_sb[:], in_=woT[:])

            nc.vector.memset(ones_sb[:], 1.0)
            nc.vector.memset(v_sb[:, :, :, DH:DH + 1], 1.0)

            # warm the PE clock while DMA streams in
            wsp = psum.tile([P, 512], F32, tag="sh", bufs=1, name="warm")
            for _ in range(8):
                nc.tensor.matmul(
                    wsp[:], ones_sb[:, 0:P], ones_sb[:],
                    start=True, stop=True,
                )

            # ---- on-device RoPE tables: sin/cos of pos*invfreq ----
            # t = ang/2pi; d = t - round(t) in [-0.5, 0.5] (int32 cast
            # rounds to nearest); sin = Sin(d, scale=2pi).  The cos path
            # shifts by +0.25 (ang + pi/2).
            I32 = mybir.dt.int32
            for qc in range(NQC):
                ssl = bass.ts(qc, 512)
                pp = psum.tile([P, 512], F32, tag="pv", bufs=1, name="posb")
                nc.tensor.matmul(
                    pp[:], ones_sb[0:1, 0:P], pos_sb[:, ssl],
                    start=True, stop=True, tile_position=(0, 0),
                )
                t_ang = tmp.tile([P, 512], F32, tag="tang", bufs=1)
                nc.vector.tensor_scalar(
                    out=t_ang[:], in0=pp[:], scalar1=invf_sb[:],
                    scalar2=None, op0=mybir.AluOpType.mult,
                )
                for dst, shift in ((sin_sb, 0.0), (cos_sb, 0.25)):
                    if shift:
                        tc_ = tmp.tile([P, 512], F32, tag="tc", bufs=1)
                        nc.vector.tensor_scalar(
                            out=tc_[:], in0=t_ang[:], scalar1=shift,
                            scalar2=None, op0=mybir.AluOpType.add,
                        )
                    else:
                        tc_ = t_ang
                    ki = tmp.tile([P, 512], I32, tag="ki", bufs=1)
                    nc.vector.tensor_copy(out=ki[:], in_=tc_[:])
                    kf = tmp.tile([P, 512], F32, tag="kf", bufs=1)
                    nc.vector.tensor_copy(out=kf[:], in_=ki[:])
                    d = tmp.tile([P, 512], F32, tag="darg", bufs=1)
                    nc.vector.tensor_tensor(
                        out=d[:], in0=tc_[:], in1=kf[:],
                        op=mybir.AluOpType.subtract,
                    )
                    # cast rounding mode differs between sim (trunc) and HW
                    # (nearest); fold d into [-0.5, 0.5) either way
                    ind = tmp.tile([P, 512], F32, tag="ind", bufs=1)
                    nc.vector.tensor_scalar(
                        out=ind[:], in0=d[:], scalar1=0.5, scalar2=None,
                        op0=mybir.AluOpType.is_ge,
                    )
                    nc.vector.tensor_tensor(
                        out=d[:], in0=d[:], in1=ind[:],
                        op=mybir.AluOpType.subtract,
                    )
                    nc.scalar.activation(
                        out=dst[:, ssl], in_=d[:],
                        func=mybir.ActivationFunctionType.Sin,
                        scale=2.0 * PI,
                    )

            # ---------------- building blocks -----------------
            def qk_chain(w_sb, rot, mt, scp, ptags=None):
                """Q/K projection chains (dt-outer over the given chunks)
                + RoPE.  rot comes out globally negated (see header)."""
                ssls = [bass.ts(sc, 512) for sc in scp]
                if ptags is None:
                    pps = [
                        psum.tile([P, 512], F32, tag="fill", bufs=2, name="pp")
                        for _ in scp
                    ]
                else:
                    pps = [
                        psum.tile([P, 1024], F32, tag=t, bufs=1, name="pp")
                        for t in ptags
                    ]
                for dt in range(8):
                    for i, sc in enumerate(scp):
                        nc.tensor.matmul(
                            pps[i][:, 0:512],
                            w_sb[:, mt, dt, :],
                            xt_sb[:, sc, dt, :],
                            start=(dt == 0),
                            stop=(dt == 7),
                        )
                for i, ssl in enumerate(ssls):
                    pp = pps[i][:, 0:512]
                    t_s = tmp.tile([P, 512], F16, tag="ts")
                    nc.vector.tensor_tensor(
                        out=t_s[:], in0=pp, in1=sin_sb[:, ssl],
                        op=mybir.AluOpType.mult,
                    )
                    sh = psum.tile([P, 512], F32, tag="sh", bufs=1, name="sh")
                    nc.tensor.matmul(
                        sh[:], st_sb[:], t_s[:], start=True, stop=True
                    )
                    nc.vector.tensor_tensor(
                        out=rot[:, mt, ssl], in0=pp, in1=cos_sb[:, ssl],
                        op=mybir.AluOpType.mult,
                    )
                    nc.vector.tensor_tensor(
                        out=rot[:, mt, ssl], in0=rot[:, mt, ssl], in1=sh[:],
                        op=mybir.AluOpType.add,
                    )

            def v_group(st):
                vp = psum.tile([P, 512], F32, tag="fill", bufs=2, name="vp")
                for dt in range(8):
                    nc.tensor.matmul(
                        vp[:, 0:256],
                        xt_sb[:, st // 4, dt, P * (st % 4):P * (st % 4 + 1)],
                        wv_sb[:, dt, :],
                        start=(dt == 0),
                        stop=(dt == 7),
                    )
                nc.vector.tensor_copy(
                    out=v_sb[:, st, :, 0:DH],
                    in_=vp[:, 0:256].rearrange("p (h d) -> p h d", h=HPG),
                )

            def scores_piece(pair, j, qa, qb, c0buf=False):
                """Scores+exp for k-tile j, both heads, query cols [qa, qb).
                c0buf: write exps into the pair-1 chunk-0 side buffer."""
                off = int(_OFF[j])
                q0 = P * j
                w = qb - qa
                tags = ("sE", "sO")
                sps = [
                    psum.tile([P, 1024], F32, tag=tags[h2], bufs=1,
                              name=tags[h2])
                    for h2 in range(2)
                ]
                for mpos in range(0, w, 512):
                    mw = min(512, w - mpos)
                    for h2 in range(2):
                        base = 64 * h2
                        nc.tensor.matmul(
                            sps[h2][:, mpos:mpos + mw],
                            krot[base:base + 64, pair, q0:q0 + P],
                            qrot[base:base + 64, pair,
                                 qa + mpos:qa + mpos + mw],
                            start=True, stop=True,
                            tile_position=(base, 0),
                        )
                for h2 in range(2):
                    if c0buf:
                        dst = ep1c0[:, h2, j, qa:qb]
                    else:
                        dst = ep_sb[:, h2, off + qa - q0:off + qb - q0]
                    nc.scalar.activation(
                        out=dst, in_=sps[h2][:, 0:w],
                        func=mybir.ActivationFunctionType.Exp,
                        scale=0.125,
                    )
                if qa == q0:
                    for h2 in range(2):
                        if c0buf:
                            blk = ep1c0[:, h2, j, qa:qa + P]
                        else:
                            blk = ep_sb[:, h2, off:off + P]
                        nc.vector.tensor_tensor(
                            out=blk, in0=blk, in1=tri_sb[:],
                            op=mybir.AluOpType.mult,
                        )

            def pv_head(pair, h2, c, c0buf=False):
                h = 2 * pair + h2
                last = 4 * c + 3
                pv = psum.tile([P, 512], F32, tag="pv", bufs=1, name="pv")
                for j in range(last + 1):
                    off = int(_OFF[j])
                    if j // 4 == c:
                        r = j % 4
                        n = 512 - P * r
                        if c0buf:
                            src = ep1c0[:, h2, j, P * r:512]
                        else:
                            src = ep_sb[:, h2, off:off + n]
                        nc.tensor.matmul(
                            pv[0:DH + 1, P * r:512],
                            v_sb[:, j, h, :],
                            src,
                            start=(j == 0), stop=(j == last),
                        )
                    else:
                        st_col = off + 512 * c - P * j
                        nc.tensor.matmul(
                            pv[0:DH + 1, :],
                            v_sb[:, j, h, :],
                            ep_sb[:, h2, st_col:st_col + 512],
                            start=(j == 0), stop=(j == last),
                        )
                pvb = tmp.tile([P, 512], F16, tag="pvb", bufs=2)
                nc.vector.tensor_copy(out=pvb[0:DH + 1, :], in_=pv[0:DH + 1, :])
                bc = psum.tile([P, 512], F32, tag="sh", bufs=1, name="bc")
                nc.tensor.matmul(
                    bc[0:DH, :],
                    ones_sb[DH:DH + 1, 0:DH],
                    pvb[DH:DH + 1, :],
                    start=True, stop=True,
                    tile_position=(64, 0),
                )
                rec = tmp.tile([P, 512], F32, tag="rec", bufs=2)
                nc.vector.reciprocal_approx_fast(
                    out=rec[0:DH, :], in_=bc[0:DH, :]
                )
                nc.vector.tensor_tensor(
                    out=at_sb[64 * h2:64 * h2 + 64, pair, bass.ts(c, 512)],
                    in0=pvb[0:DH, :], in1=rec[0:DH, :],
                    op=mybir.AluOpType.mult,
                )

            def outproj_piece(c, ot):
                ssl = bass.ts(c, 512)
                osl = bass.ts(ot, P)
                po = psum.tile([P, 512], F32, tag="fill", bufs=2, name="po")
                for ct in range(2):
                    nc.tensor.matmul(
                        po[:],
                        wo_sb[:, ct, osl],
                        at_sb[:, ct, ssl],
                        start=(ct == 0), stop=(ct == 1),
                    )
                ob = outp.tile([P, 512], F16, tag="ob")
                nc.vector.tensor_copy(out=ob[:], in_=po[:])
                if ot % 2 == 0:
                    nc.scalar.dma_start(out=outT[ot, c], in_=ob[:])
                else:
                    nc.gpsimd.dma_start(out=outT[ot, c], in_=ob[:])

            # ---------------- schedule -----------------
            # Event kinds: s=(pair,j,qa,qb[,c0buf]) scores; pv=(pair,h2,c
            # [,c0buf]); qk=(w,mt,chunks); v=(st,); op=(c,) queue outproj.
            th0 = [
                ("qk", "q", 0, (0,)), ("qk", "k", 0, (0,)),
                ("s", 0, 0, 0, 512), ("s", 0, 1, 128, 512),
                ("s", 0, 2, 256, 512), ("s", 0, 3, 384, 512),
                ("qk", "q", 1, (0,)), ("qk", "k", 1, (0,)),
                ("sc0", 1, 0, 0, 512), ("sc0", 1, 1, 128, 512),
                ("sc0", 1, 2, 256, 512), ("sc0", 1, 3, 384, 512),
                ("v", 0), ("v", 1), ("v", 2), ("v", 3),
                ("pv", 0, 0, 0), ("pv", 0, 1, 0),
                ("pvc0", 1, 0, 0), ("pvc0", 1, 1, 0),
                ("op", 0),
                ("qk", "q", 0, (1,)), ("qk", "k", 0, (1,)),
                ("s", 0, 0, 512, 1024), ("s", 0, 1, 512, 1024),
                ("s", 0, 2, 512, 1024), ("s", 0, 3, 512, 1024),
                ("s", 0, 4, 512, 1024), ("v", 4),
                ("s", 0, 5, 640, 1024), ("v", 5),
                ("s", 0, 6, 768, 1024), ("v", 6),
                ("s", 0, 7, 896, 1024), ("v", 7),
                ("qk", "q", 1, (1,)), ("qk", "k", 1, (1,)),
                ("pv", 0, 0, 1), ("pv", 0, 1, 1),
                ("qk", "q", 0, (2, 3)),
                ("s", 0, 0, 1024, 2048), ("s", 0, 1, 1024, 2048),
                ("qk", "k", 0, (2, 3)),
                ("s", 0, 2, 1024, 2048), ("s", 0, 3, 1024, 2048),
                ("s", 0, 8, 1024, 2048), ("v", 8),
                ("s", 0, 9, 1152, 2048), ("v", 9),
                ("s", 0, 10, 1280, 2048), ("v", 10),
                ("s", 0, 11, 1408, 2048), ("v", 11),
                ("s", 0, 4, 1024, 2048), ("qk", "q", 1, (2, 3)),
                ("s", 0, 5, 1024, 2048),
                ("s", 0, 6, 1024, 2048), ("qk", "k", 1, (2, 3)),
                ("s", 0, 7, 1024, 2048),
                ("pv", 0, 0, 2), ("pv", 0, 1, 2),
                ("s", 0, 12, 1536, 2048), ("v", 12),
                ("s", 0, 13, 1664, 2048), ("v", 13),
                ("s", 0, 14, 1792, 2048), ("v", 14),
                ("s", 0, 15, 1920, 2048), ("v", 15),
                ("pv", 0, 0, 3), ("pv", 0, 1, 3),
            ]
            th1 = [
                ("s", 1, 0, 512, 1024), ("s", 1, 1, 512, 1024),
                ("s", 1, 2, 512, 1024), ("s", 1, 3, 512, 1024),
                ("s", 1, 4, 512, 1024),
                ("s", 1, 5, 640, 1024),
                ("s", 1, 6, 768, 1024),
                ("s", 1, 7, 896, 1024),
                ("pv", 1, 0, 1), ("pv", 1, 1, 1), ("op", 1),
                ("s", 1, 0, 1024, 2048), ("s", 1, 1, 1024, 2048),
                ("s", 1, 2, 1024, 2048), ("s", 1, 3, 1024, 2048),
                ("s", 1, 8, 1024, 2048),
                ("s", 1, 9, 1152, 2048),
                ("s", 1, 10, 1280, 2048),
                ("s", 1, 11, 1408, 2048),
                ("s", 1, 4, 1024, 2048), ("s", 1, 5, 1024, 2048),
                ("s", 1, 6, 1024, 2048), ("s", 1, 7, 1024, 2048),
                ("pv", 1, 0, 2), ("pv", 1, 1, 2), ("op", 2),
                ("s", 1, 12, 1536, 2048), ("s", 1, 13, 1664, 2048),
                ("s", 1, 14, 1792, 2048), ("s", 1, 15, 1920, 2048),
                ("pv", 1, 0, 3), ("pv", 1, 1, 3), ("op", 3),
            ]

            ready_out = []

            def weave(n):
                k = 0
                while k < n and ready_out:
                    ready_out.pop(0)()
                    k += 1

            for ev in th0 + th1:
                kind = ev[0]
                if kind in ("s", "sc0"):
                    _, pair, j, qa, qb = ev
                    pos = qa
                    while pos < qb:
                        w = min(1024, qb - pos)
                        scores_piece(pair, j, pos, pos + w,
                                     c0buf=(kind == "sc0"))
                        pos += w
                        weave(2)
                elif kind in ("pv", "pvc0"):
                    _, pair, h2, c = ev
                    pv_head(pair, h2, c, c0buf=(kind == "pvc0"))
                    weave(1)
                elif kind == "op":
                    ready_out.extend(
                        [lambda c=ev[1], ot=ot: outproj_piece(c, ot)
                         for ot in range(8)]
                    )
                    weave(2)
                elif kind == "qk":
                    _, w, mt, scp = ev
                    tgt = (wq_sb, qrot) if w == "q" else (wk_sb, krot)
                    qk_chain(tgt[0], tgt[1], mt, scp)
                elif kind == "v":
                    v_group(ev[1])
            while ready_out:
                ready_out.pop(0)()

    nc.compile()
    return nc


_PROGRAM = None


def _get_program():
    global _PROGRAM
    if _PROGRAM is None:
        _PROGRAM = build_program()
    return _PROGRAM


def _host_consts(token_positions):
    pos = np.asarray(token_positions, dtype=np.float32)
    inv = (
        ROPE_THETA ** (-np.arange(0, DH, 2, dtype=np.float32) / DH)
    ).astype(np.float32)
    rows = (np.arange(P) % DH) // 2
    invf = np.ascontiguousarray(
        (inv[rows] / (2.0 * np.pi)).reshape(P, 1)
    ).astype(np.float32)
    posf = np.ascontiguousarray(pos.reshape(1, S)).astype(np.float16)
    Smat = np.zeros((P, P), dtype=np.float32)
    idx = np.arange(0, P, 2)
    Smat[idx, idx + 1] = -1.0
    Smat[idx + 1, idx] = 1.0
    ST = np.ascontiguousarray(Smat.T).astype(np.float16)
    tri = (np.arange(P)[None, :] >= np.arange(P)[:, None]).astype(np.float16)
    return posf, invf, ST, tri


def _make_in_maps(x, W_q, W_k, W_v, W_o, token_positions):
    posf, invf, ST, tri = _host_consts(token_positions)
    x = np.asarray(x, dtype=np.float32)
    maps = []
    for core in range(NCORE):
        b, hg = core // 4, core % 4
        hsl = slice(256 * hg, 256 * (hg + 1))
        xT = x[b].T.astype(np.float16)                      # [1024, 2048]
        xts = np.ascontiguousarray(
            xT.reshape(8, P, NQC, 512).transpose(2, 1, 0, 3)
        )                                                   # [4, 128, 8, 512]
        wqT = np.asarray(W_q, np.float32)[hsl].T            # [1024, 256]
        wkT = np.asarray(W_k, np.float32)[hsl].T
        wq2 = np.ascontiguousarray(
            wqT.reshape(8, P, 2, P).transpose(2, 1, 0, 3)
        ).astype(np.float16)                                # [2, 128, 8, 128]
        wk2 = np.ascontiguousarray(
            wkT.reshape(8, P, 2, P).transpose(2, 1, 0, 3)
        ).astype(np.float16)
        wvs = np.asarray(W_v, np.float32)[hsl].T
        wvs = np.ascontiguousarray(
            wvs.reshape(8, P, 256).transpose(1, 0, 2)
        ).astype(np.float16)                                # [128, 8, 256]
        wo_p = np.asarray(W_o, dtype=np.float32)[:, hsl].T
        wo_p = np.ascontiguousarray(
            wo_p.reshape(2, P, D).transpose(1, 0, 2)
        ).astype(np.float16)                                # [128, 2, 1024]
        maps.append(
            {
                "xts": xts,
                "wq2": wq2,
                "wk2": wk2,
                "wvT": wvs,
                "woT": wo_p,
                "posf": posf,
                "invf": invf,
                "ST": ST,
                "trimask": tri,
            }
        )
    return maps


def _assemble(results):
    out = np.zeros((B, S, D), dtype=np.float32)
    for core in range(NCORE):
        b = core // 4
        blk = results[core]["outT"].astype(np.float32)  # [8, 4, 128, 512]
        full = blk.transpose(0, 2, 1, 3).reshape(D, S)
        out[b] += full.T
    return out


def _run(in_maps, trace=False):
    nc = _get_program()
    tmpdir = None
    if trace:
        import tempfile

        tmpdir = tempfile.mkdtemp(prefix="ntff_", dir="/tmp")
    res = run_bass_kernel_spmd(
        nc, in_maps, list(range(NCORE)), trace=trace, tmpdir=tmpdir
    )
    return res


def kernel(x, W_q, W_k, W_v, W_o, token_positions):
    in_maps = _make_in_maps(x, W_q, W_k, W_v, W_o, token_positions)
    res = _run(in_maps)
    return _assemble(res.results)


def _install_profile_hook():
    """The agent image's antenv lacks axon_hooks; shim it so trace=True works."""
    import sys
    import types

    try:
        from antenv.axon_hooks import get_axon_ntff_profile_hook  # noqa: F401
        return
    except ImportError:
        pass
    import antenv
    from trn_agent_boot.trn_boot import _ntff_profile_via_ctypes

    mod = types.ModuleType("antenv.axon_hooks")
    _hook = {"h": None}
    mod.set_axon_ntff_profile_hook = lambda h: _hook.__setitem__("h", h)
    mod.get_axon_ntff_profile_hook = lambda: _hook["h"]
    sys.modules["antenv.axon_hooks"] = mod
    antenv.axon_hooks = mod
    mod.set_axon_ntff_profile_hook(
        _ntff_profile_via_ctypes("/opt/axon/libaxon_pjrt.so")
    )
    import concourse.bass_utils as bu

    bu.upload_artifacts = lambda d: f"file://{d}"


def kernel_traced(x, W_q, W_k, W_v, W_o, token_positions):
    """Returns (output, exec_time_ns, trace_path)."""
    _install_profile_hook()
    in_maps = _make_in_maps(x, W_q, W_k, W_v, W_o, token_positions)
    res = _run(in_maps, trace=True)
    trace_path = None
    if res.instructions_and_trace is not None:
        trace_path = res.instructions_and_trace[1]
    return _assemble(res.results), res.exec_time_ns, trace_path
